# revision 3
# baseline (speedup 1.0000x reference)
"""Multi-head attention on 8 Trainium2 NeuronCores.

Sharding: core c = (batch n, head-group g); n = c // 4, g = c % 4.
Each core computes attention for its 4 heads of its batch entry plus the
fc_out partial product for those heads' rows of Wo (with bo/4 folded in);
the host sums the 4 partials per batch to unshard.

Per-core pipeline (all matmuls bf16, accumulation f32 in PSUM):
  A) qT/kT projections head-pair-stacked ([d,L] layout, pair in partition
     halves 0-63 / 64-127), v projection in [k, d] layout with a ones
     column appended per head (gives the softmax denominator for free).
  B) scoresT = kT.T-style matmul ([k, q] layout, K=64 row-tiled pairs so
     both heads of a pair run concurrently in the PE array), exp on
     ScalarE straight out of PSUM (scale=1/32; no max subtraction needed:
     scores ~ N(0, 1/16)), attn@v accumulated over k tiles into [d+1, q]
     PSUM (row 64 = denominator).
  C) bulk reciprocal of all denominators, DMA partition-broadcast,
     normalize, then fc_out partial = Wo_slice.T @ outT with K=65 (row 64
     of the stationary = bo/4 for local head 0, zeros otherwise).
"""

import os
import sys

for _p in ("/opt/trn_rl_repo",):
    if _p not in sys.path and os.path.isdir(_p):
        sys.path.insert(0, _p)

import numpy as np
import ml_dtypes

import concourse.bass as bass
import concourse.mybir as mybir
import concourse.tile as tile
from concourse import bacc
from concourse.bass import ds, ts
from concourse.bass_utils import run_bass_kernel_spmd

BF16 = ml_dtypes.bfloat16
F32 = np.float32

EMBED = 1024
HEADS = 16
HD = 64  # head dim
NB = 2  # batch
L = 2048  # sequence length
NCORES = 8
HPG = 4  # heads per group (per core)
NPAIRS = 2  # head pairs per core
ET = EMBED // 128  # 8 contraction tiles for projections
LT = L // 128  # 16 k tiles
QS = 1024  # q superchunk (exp free-dim)
NQS = L // QS  # 2
NLC = L // 512  # 4 512-wide l chunks

SCALE = 1.0 / np.sqrt(np.float32(EMBED))  # 1/32

LAST_EXEC_TIME_NS = None
LAST_RESULTS = None

_nc_cache = None


def build_nc():
    """Build + compile the per-core Bass program (same program on all cores)."""
    nc = bacc.Bacc("TRN2")
    f32 = mybir.dt.float32
    bf16 = mybir.dt.bfloat16
    EXP = mybir.ActivationFunctionType.Exp

    xT_d = nc.declare_dram_parameter("xT", [EMBED, L], bf16, isOutput=False)
    wqk_d = nc.declare_dram_parameter("wqk", [4, EMBED, 128], bf16, isOutput=False)
    wv_d = nc.declare_dram_parameter("wv", [EMBED, HPG * HD], bf16, isOutput=False)
    wo_d = nc.declare_dram_parameter("wo", [HPG, ET, HD + 1, 128], bf16, isOutput=False)
    ones_d = nc.declare_dram_parameter("ones", [1, HPG * L], bf16, isOutput=False)
    out_d = nc.declare_dram_parameter("out", [EMBED, L], f32, isOutput=True)
    recip_dram = nc.dram_tensor("recip_dram", [16, 512], f32)

    with tile.TileContext(nc) as tc:
        with (
            tc.tile_pool(name="singles", bufs=1) as singles,
            tc.tile_pool(name="expp", bufs=4) as expp,
            tc.tile_pool(name="drowp", bufs=4) as drowp,
            tc.tile_pool(name="rbp", bufs=4) as rbp,
            tc.tile_pool(name="outp", bufs=4) as outp,
        ):
            # ---- resident SBUF tensors ----
            xT_sb = singles.tile([128, ET, L], bf16, name="xT_sb")
            wqk_sb = singles.tile([128, 4, ET, 128], bf16, name="wqk_sb")
            wv_sb = singles.tile([128, ET, HPG * HD], bf16, name="wv_sb")
            wo_sb = singles.tile([HD + 1, HPG, ET, 128], bf16, name="wo_sb")
            qt_sb = singles.tile([128, NPAIRS, L], bf16, name="qt_sb")
            kt_sb = singles.tile([128, NPAIRS, L], bf16, name="kt_sb")
            v_sb = singles.tile([128, LT, HPG, HD + 1], bf16, name="v_sb")
            outT_sb = singles.tile([HD + 1, HPG, L], bf16, name="outT_sb")
            num_sb = singles.tile([HD, HPG, L], f32, name="num_sb")
            denom_sb = singles.tile([16, 512], f32, name="denom_sb")
            recip_sb = singles.tile([16, 512], f32, name="recip_sb")

            # ---- input DMAs ----
            xT_ap = xT_d[:].rearrange("(t p) l -> p t l", p=128)
            for et in range(ET):
                nc.sync.dma_start(out=xT_sb[:, et, :], in_=xT_ap[:, et, :])
            nc.sync.dma_start(
                out=wqk_sb, in_=wqk_d[:].rearrange("j (t p) c -> p j t c", p=128)
            )
            nc.sync.dma_start(
                out=wv_sb, in_=wv_d[:].rearrange("(t p) c -> p t c", p=128)
            )
            nc.sync.dma_start(
                out=wo_sb, in_=wo_d[:].rearrange("h t p c -> p h t c")
            )
            # ones row of outT (row 64) - the bias contraction row for fc_out
            nc.sync.dma_start(
                out=outT_sb[HD : HD + 1, :, :],
                in_=ones_d[:].rearrange("o (h l) -> o h l", h=HPG),
            )

            # ================= Phase A: projections =================
            with tc.tile_pool(name="psA", bufs=8, space="PSUM") as psA:
                # qT/kT, head-pair stacked: j = pair*2 + (0 for q, 1 for k)
                for j in range(4):
                    pair, qk = divmod(j, 2)
                    pst = [
                        psA.tile([128, 512], f32, tag="ps", name=f"qk{j}_{lc}")
                        for lc in range(NLC)
                    ]
                    for et in range(ET):
                        for lc in range(NLC):
                            nc.tensor.matmul(
                                pst[lc],
                                wqk_sb[:, j, et, :],
                                xT_sb[:, et, ts(lc, 512)],
                                start=(et == 0),
                                stop=(et == ET - 1),
                            )
                    dst = qt_sb if qk == 0 else kt_sb
                    for lc in range(NLC):
                        nc.vector.tensor_copy(dst[:, pair, ts(lc, 512)], pst[lc])

                # v in [k, d] layout, 4 heads side by side
                for lt in range(LT):
                    pv = psA.tile([128, 512], f32, tag="ps", name=f"v{lt}")
                    pv = pv[:, : HPG * HD]
                    for et in range(ET):
                        nc.tensor.matmul(
                            pv,
                            xT_sb[:, et, ts(lt, 128)],
                            wv_sb[:, et, :],
                            start=(et == 0),
                            stop=(et == ET - 1),
                        )
                    nc.vector.tensor_copy(
                        v_sb[:, lt, :, 0:HD],
                        pv.rearrange("p (h d) -> p h d", h=HPG),
                    )
                # ones column per head (denominator accumulator)
                nc.vector.memset(v_sb[:, :, :, HD : HD + 1], 1.0)

            # ================= Phase B: attention =================
            ridx = 0
            rmap = []  # ridx -> (h_local, col0)
            with (
                tc.tile_pool(name="psS", bufs=2, space="PSUM") as psS,
                tc.tile_pool(name="psAV", bufs=4, space="PSUM") as psAV,
            ):
                for pair in range(NPAIRS):
                    for qs in range(NQS):
                        av = {}
                        for side in range(2):
                            for half in range(2):
                                av[(side, half)] = psAV.tile(
                                    [128, 512],
                                    f32,
                                    tag="av",
                                    name=f"av{pair}{qs}{side}{half}",
                                )
                        for k in range(LT):
                            for side in range(2):
                                base = side * HD
                                h_local = pair * 2 + side
                                sc = psS.tile(
                                    [128, QS], f32, tag="sc", name=f"sc{side}"
                                )
                                for half in range(2):
                                    nc.tensor.matmul(
                                        sc[:, ts(half, 512)],
                                        kt_sb[base : base + HD, pair, ts(k, 128)],
                                        qt_sb[
                                            base : base + HD,
                                            pair,
                                            ds(qs * QS + half * 512, 512),
                                        ],
                                        start=True,
                                        stop=True,
                                    )
                                ex = expp.tile([128, QS], bf16, tag="exp", name="ex")
                                nc.scalar.activation(ex, sc, EXP, scale=float(SCALE))
                                for half in range(2):
                                    nc.tensor.matmul(
                                        av[(side, half)][0 : HD + 1, :],
                                        v_sb[:, k, h_local, :],
                                        ex[:, ts(half, 512)],
                                        start=(k == 0),
                                        stop=(k == LT - 1),
                                    )
                        # evacuate numerators + denominator rows
                        for side in range(2):
                            h_local = pair * 2 + side
                            for half in range(2):
                                avt = av[(side, half)]
                                col0 = qs * QS + half * 512
                                nc.vector.tensor_copy(
                                    num_sb[:, h_local, ds(col0, 512)], avt[0:HD, :]
                                )
                                dr = drowp.tile(
                                    [HD + 1, 512], f32, tag="dr", name="dr"
                                )
                                nc.vector.tensor_copy(
                                    dr[HD : HD + 1, :], avt[HD : HD + 1, :]
                                )
                                nc.sync.dma_start(
                                    out=denom_sb[ridx : ridx + 1, :],
                                    in_=dr[HD : HD + 1, :],
                                )
                                rmap.append((h_local, col0))
                                ridx += 1

            # ---- bulk reciprocal + normalize ----
            nc.vector.reciprocal(recip_sb, denom_sb)
            # partition-broadcast needs a DRAM source; bounce through DRAM
            nc.sync.dma_start(out=recip_dram[:], in_=recip_sb)
            for j, (h_local, col0) in enumerate(rmap):
                rb = rbp.tile([HD, 512], f32, tag="rb", name="rb")
                nc.sync.dma_start(
                    out=rb, in_=recip_dram[j : j + 1, :].to_broadcast([HD, 512])
                )
                nc.vector.tensor_mul(
                    outT_sb[0:HD, h_local, ds(col0, 512)],
                    num_sb[:, h_local, ds(col0, 512)],
                    rb,
                )

            # ================= Phase C: fc_out partial =================
            with tc.tile_pool(name="psC", bufs=8, space="PSUM") as psC:
                for et in range(ET):
                    fps = [
                        psC.tile([128, 512], f32, tag="fc", name=f"fc{et}_{lc}")
                        for lc in range(NLC)
                    ]
                    for lh in range(HPG):
                        for lc in range(NLC):
                            nc.tensor.matmul(
                                fps[lc],
                                wo_sb[:, lh, et, :],
                                outT_sb[:, lh, ts(lc, 512)],
                                start=(lh == 0),
                                stop=(lh == HPG - 1),
                            )
                    for lc in range(NLC):
                        ob = outp.tile([128, 512], f32, tag="ob", name="ob")
                        nc.vector.tensor_copy(ob, fps[lc])
                        nc.sync.dma_start(
                            out=out_d[ts(et, 128), ts(lc, 512)], in_=ob
                        )

    nc.compile()
    return nc


def get_nc():
    global _nc_cache
    if _nc_cache is None:
        _nc_cache = build_nc()
    return _nc_cache


def make_core_inputs(x, Wq, Wk, Wv, Wo, bo):
    """Build the 8 per-core input maps from the full-size inputs."""
    x = np.asarray(x, F32)
    Wq = np.asarray(Wq, F32)
    Wk = np.asarray(Wk, F32)
    Wv = np.asarray(Wv, F32)
    Wo = np.asarray(Wo, F32)
    bo = np.asarray(bo, F32)

    xT_b = [np.ascontiguousarray(x[n].T).astype(BF16) for n in range(NB)]
    ones = np.ones((1, HPG * L), BF16)

    in_maps = []
    for c in range(NCORES):
        n, g = divmod(c, HPG)
        heads = [g * HPG + i for i in range(HPG)]

        wqk = np.empty((4, EMBED, 128), F32)
        for j in range(4):
            pair, qk = divmod(j, 2)
            hA = heads[2 * pair]
            hB = heads[2 * pair + 1]
            W = Wq if qk == 0 else Wk
            wqk[j, :, 0:HD] = W[hA * HD : (hA + 1) * HD, :].T
            wqk[j, :, HD:128] = W[hB * HD : (hB + 1) * HD, :].T

        wv = np.concatenate(
            [Wv[h * HD : (h + 1) * HD, :].T for h in heads], axis=1
        )  # [1024, 256]

        wo = np.zeros((HPG, ET, HD + 1, 128), F32)
        for lh, h in enumerate(heads):
            for et in range(ET):
                wo[lh, et, 0:HD, :] = Wo[
                    et * 128 : (et + 1) * 128, h * HD : (h + 1) * HD
                ].T
        for et in range(ET):
            wo[0, et, HD, :] = bo[et * 128 : (et + 1) * 128] / HPG

        in_maps.append(
            {
                "xT": xT_b[n],
                "wqk": wqk.astype(BF16),
                "wv": wv.astype(BF16),
                "wo": wo.astype(BF16),
                "ones": ones,
            }
        )
    return in_maps


def combine_outputs(results):
    """Sum the per-core fc_out partials and transpose back to [N, L, E]."""
    out = np.empty((NB, L, EMBED), F32)
    for n in range(NB):
        acc = results[n * HPG]["out"].astype(F32).copy()
        for g in range(1, HPG):
            acc += results[n * HPG + g]["out"]
        out[n] = acc.T
    return out


def kernel(x, Wq, Wk, Wv, Wo, bo):
    global LAST_EXEC_TIME_NS, LAST_RESULTS
    nc = get_nc()
    in_maps = make_core_inputs(x, Wq, Wk, Wv, Wo, bo)
    trace = bool(os.environ.get("KERNEL_TRACE"))
    kw = {}
    if trace:
        kw["trace"] = True
        kw["trace_cores"] = list(range(NCORES))
    res = run_bass_kernel_spmd(nc, in_maps, list(range(NCORES)), **kw)
    LAST_EXEC_TIME_NS = res.exec_time_ns
    LAST_RESULTS = res
    return combine_outputs(res.results)


# revision 11
# speedup vs baseline: 1.3413x; 1.3413x over previous
"""Multi-head attention on 8 Trainium2 NeuronCores.

Sharding: core c = (batch n, head-group g); n = c // 4, g = c % 4.
Each core computes attention for its 4 heads of its batch entry plus the
fc_out partial product for those heads' rows of Wo (with bo/4 folded in);
the host sums the 4 partials per batch to unshard.

Per-core pipeline (all matmuls bf16, accumulation f32 in PSUM):
  A) qT/kT projections head-pair-stacked ([d,L] layout, pair in partition
     halves 0-63 / 64-127), v projection in [k, d] layout with a ones
     column appended per head (gives the softmax denominator for free).
  B) scoresT = kT.T-style matmul ([k, q] layout, K=64 row-tiled pairs so
     both heads of a pair run concurrently in the PE array), exp on
     ScalarE straight out of PSUM (scale=1/32; no max subtraction needed:
     scores ~ N(0, 1/16)), attn@v accumulated over k tiles into [d+1, q]
     PSUM (row 64 = denominator).
  C) bulk reciprocal of all denominators, DMA partition-broadcast,
     normalize, then fc_out partial = Wo_slice.T @ outT with K=65 (row 64
     of the stationary = bo/4 for local head 0, zeros otherwise).
"""

import contextlib as _contextlib
import os
import sys

for _p in ("/opt/trn_rl_repo",):
    if _p not in sys.path and os.path.isdir(_p):
        sys.path.insert(0, _p)

import numpy as np
import ml_dtypes

import concourse.bass as bass
import concourse.mybir as mybir
import concourse.tile as tile
from concourse import bacc
from concourse.bass import ds, ts
from concourse.bass_utils import run_bass_kernel_spmd

BF16 = ml_dtypes.bfloat16
F32 = np.float32

EMBED = 1024
HEADS = 16
HD = 64  # head dim
NB = 2  # batch
L = 2048  # sequence length
NCORES = 8
HPG = 4  # heads per group (per core)
NPAIRS = 2  # head pairs per core
ET = EMBED // 128  # 8 contraction tiles for projections
LT = L // 128  # 16 k tiles
QS = 1024  # q superchunk (exp free-dim)
NQS = L // QS  # 2
NLC = L // 512  # 4 512-wide l chunks

SCALE = 1.0 / np.sqrt(np.float32(EMBED))  # 1/32

LAST_EXEC_TIME_NS = None
LAST_RESULTS = None

_nc_cache = None


def build_nc():
    """Build + compile the per-core Bass program (same program on all cores)."""
    nc = bacc.Bacc("TRN2")
    f32 = mybir.dt.float32
    bf16 = mybir.dt.bfloat16
    EXP = mybir.ActivationFunctionType.Exp

    xT_d = nc.declare_dram_parameter("xT", [EMBED, L], bf16, isOutput=False)
    wqk_d = nc.declare_dram_parameter("wqk", [4, EMBED, 128], bf16, isOutput=False)
    wv_d = nc.declare_dram_parameter("wv", [EMBED, HPG * HD], bf16, isOutput=False)
    wo_d = nc.declare_dram_parameter("wo", [HPG, ET, HD + 1, 128], bf16, isOutput=False)
    ones_d = nc.declare_dram_parameter("ones", [1, HPG * L], bf16, isOutput=False)
    out_d = nc.declare_dram_parameter("out", [EMBED, L], f32, isOutput=True)
    recip_dram = nc.dram_tensor("recip_dram", [16, 512], bf16)

    with tile.TileContext(nc) as tc:
        with (
            tc.tile_pool(name="singles", bufs=1) as singles,
            tc.tile_pool(name="expp", bufs=20) as expp,
            tc.tile_pool(name="drowp", bufs=4) as drowp,
            tc.tile_pool(name="rbp", bufs=6) as rbp,
            tc.tile_pool(name="outp", bufs=4) as outp,
        ):
            # ---- resident SBUF tensors ----
            xT_sb = singles.tile([128, ET, L], bf16, name="xT_sb")
            wqk_sb = singles.tile([128, 4, ET, 128], bf16, name="wqk_sb")
            wv_sb = singles.tile([128, ET, HPG * HD], bf16, name="wv_sb")
            wo_sb = singles.tile([HD + 1, HPG, ET, 128], bf16, name="wo_sb")
            qt_sb = singles.tile([128, NPAIRS, L], bf16, name="qt_sb")
            kt_sb = singles.tile([128, NPAIRS, L], bf16, name="kt_sb")
            v_sb = singles.tile([128, LT, HPG, HD + 1], bf16, name="v_sb")
            outT_sb = singles.tile([HD + 1, HPG, L], bf16, name="outT_sb")
            num_sb = singles.tile([HD, HPG, L], bf16, name="num_sb")
            # per-(pair,qs) denominator blocks: 4 rows each, base partition 0
            denom_bl = [
                singles.tile([4, 512], f32, name=f"denom{b}") for b in range(4)
            ]
            recip_bl = [
                singles.tile([4, 512], f32, name=f"recip{b}") for b in range(4)
            ]
            recipb_bl = [
                singles.tile([4, 512], bf16, name=f"recipb{b}") for b in range(4)
            ]

            # ---- input DMAs (ordered so compute can start early) ----
            xT_ap = xT_d[:].rearrange("(t p) l -> p t l", p=128)
            wqk_ap = wqk_d[:].rearrange("j (t p) c -> p j t c", p=128)
            for et in range(ET):
                nc.sync.dma_start(out=xT_sb[:, et, :], in_=xT_ap[:, et, :])
            for j in range(4):
                nc.sync.dma_start(out=wqk_sb[:, j, :, :], in_=wqk_ap[:, j, :, :])
            nc.sync.dma_start(
                out=wv_sb, in_=wv_d[:].rearrange("(t p) c -> p t c", p=128)
            )
            nc.sync.dma_start(out=wo_sb, in_=wo_d[:].rearrange("h t p c -> p h t c"))
            # ones row of outT (row 64) - the bias contraction row for fc_out
            nc.sync.dma_start(
                out=outT_sb[HD : HD + 1, :, :],
                in_=ones_d[:].rearrange("o (h l) -> o h l", h=HPG),
            )

            # scores PSUM pool spans phases A+B only; closed before fc so
            # its banks are free for psC
            _psS_stack = _contextlib.ExitStack()
            psS = _psS_stack.enter_context(
                tc.tile_pool(name="psS", bufs=2, space="PSUM")
            )

            ex_store = {}  # (pair, qs, side, k) -> exp tile emitted early

            def emit_sc_exp(pair, qs, side, k):
                base = side * HD
                sc = psS.tile([128, QS], f32, tag="sc", name=f"sc{side}")
                for half in range(2):
                    nc.tensor.matmul(
                        sc[:, ts(half, 512)],
                        kt_sb[base : base + HD, pair, ts(k, 128)],
                        qt_sb[base : base + HD, pair, ds(qs * QS + half * 512, 512)],
                        start=True,
                        stop=True,
                    )
                ex = expp.tile([128, QS], bf16, tag="exp", name="ex")
                nc.scalar.activation(ex, sc, EXP, scale=float(SCALE))
                return ex

            # ================= Phase A: projections =================
            # early-emit list: scores+exp for (pair0, qs0, side0) woven into
            # the j2/j3 projection loops so ScalarE starts ~35us earlier
            early = [(0, 0, 0, k) for k in range(LT)]

            def emit_qk_group(j, interleave):
                pair, qk = divmod(j, 2)
                pst = [
                    psA.tile([128, 512], f32, tag="ps", name=f"qk{j}_{lc}")
                    for lc in range(NLC)
                ]
                for et in range(ET):
                    for lc in range(NLC):
                        nc.tensor.matmul(
                            pst[lc],
                            wqk_sb[:, j, et, :],
                            xT_sb[:, et, ts(lc, 512)],
                            start=(et == 0),
                            stop=(et == ET - 1),
                        )
                    if interleave and early:
                        key = early.pop(0)
                        ex_store[key] = emit_sc_exp(*key)
                dst = qt_sb if qk == 0 else kt_sb
                for lc in range(NLC):
                    nc.vector.tensor_copy(dst[:, pair, ts(lc, 512)], pst[lc])

            with tc.tile_pool(name="psA", bufs=4, space="PSUM") as psA:
                emit_qk_group(0, False)
                emit_qk_group(1, False)
                # v in [k, d] layout, 4 heads side by side
                for lt in range(LT):
                    pv = psA.tile([128, 512], f32, tag="ps", name=f"v{lt}")
                    pv = pv[:, : HPG * HD]
                    for et in range(ET):
                        nc.tensor.matmul(
                            pv,
                            xT_sb[:, et, ts(lt, 128)],
                            wv_sb[:, et, :],
                            start=(et == 0),
                            stop=(et == ET - 1),
                        )
                    nc.vector.tensor_copy(
                        v_sb[:, lt, :, 0:HD],
                        pv.rearrange("p (h d) -> p h d", h=HPG),
                    )
                nc.vector.memset(v_sb[:, :, :, HD : HD + 1], 1.0)
                emit_qk_group(2, True)
                emit_qk_group(3, True)

            # ================= Phase B: attention =================
            ridx = 0
            with tc.tile_pool(name="psAV", bufs=4, space="PSUM") as psAV:
                for pair in range(NPAIRS):
                    for qs in range(NQS):
                        av = {}
                        for side in range(2):
                            for half in range(2):
                                av[(side, half)] = psAV.tile(
                                    [128, 512],
                                    f32,
                                    tag="av",
                                    name=f"av{pair}{qs}{side}{half}",
                                )
                        for k in range(LT):
                            for side in range(2):
                                h_local = pair * 2 + side
                                key = (pair, qs, side, k)
                                if key in ex_store:
                                    ex = ex_store.pop(key)
                                else:
                                    ex = emit_sc_exp(pair, qs, side, k)
                                for half in range(2):
                                    nc.tensor.matmul(
                                        av[(side, half)][0 : HD + 1, :],
                                        v_sb[:, k, h_local, :],
                                        ex[:, ts(half, 512)],
                                        start=(k == 0),
                                        stop=(k == LT - 1),
                                    )
                        # inline normalize for this (pair, qs):
                        # evacuate numerators + denominator rows
                        blk = pair * NQS + qs
                        r0 = ridx
                        for side in range(2):
                            h_local = pair * 2 + side
                            for half in range(2):
                                avt = av[(side, half)]
                                col0 = qs * QS + half * 512
                                nc.vector.tensor_copy(
                                    num_sb[:, h_local, ds(col0, 512)], avt[0:HD, :]
                                )
                                dr = drowp.tile(
                                    [HD + 1, 512], f32, tag="dr", name="dr"
                                )
                                nc.vector.tensor_copy(
                                    dr[HD : HD + 1, :], avt[HD : HD + 1, :]
                                )
                                nc.sync.dma_start(
                                    out=denom_bl[blk][ridx - r0 : ridx - r0 + 1, :],
                                    in_=dr[HD : HD + 1, :],
                                )
                                ridx += 1
                        # reciprocal for these 4 rows, bounce via DRAM for the
                        # partition-broadcast, then normalize into outT
                        nc.vector.reciprocal(recip_bl[blk], denom_bl[blk])
                        nc.vector.tensor_copy(recipb_bl[blk], recip_bl[blk])
                        nc.sync.dma_start(
                            out=recip_dram[r0:ridx, :], in_=recipb_bl[blk]
                        )
                        j = r0
                        for side in range(2):
                            h_local = pair * 2 + side
                            for half in range(2):
                                col0 = qs * QS + half * 512
                                rb = rbp.tile([HD, 512], bf16, tag="rb", name="rb")
                                nc.sync.dma_start(
                                    out=rb,
                                    in_=recip_dram[j : j + 1, :].to_broadcast(
                                        [HD, 512]
                                    ),
                                )
                                nc.vector.tensor_mul(
                                    outT_sb[0:HD, h_local, ds(col0, 512)],
                                    num_sb[0:HD, h_local, ds(col0, 512)],
                                    rb,
                                )
                                j += 1

            _psS_stack.close()  # free scores banks before fc

            # ================= Phase C: fc_out partial =================
            with tc.tile_pool(name="psC", bufs=8, space="PSUM") as psC:
                for lc in range(NLC):
                    for et in range(ET):
                        fps = psC.tile(
                            [128, 512], f32, tag="fc", name=f"fc{et}_{lc}"
                        )
                        for lh in range(HPG):
                            nc.tensor.matmul(
                                fps,
                                wo_sb[:, lh, et, :],
                                outT_sb[:, lh, ts(lc, 512)],
                                start=(lh == 0),
                                stop=(lh == HPG - 1),
                            )
                        ob = outp.tile([128, 512], f32, tag="ob", name="ob")
                        nc.vector.tensor_copy(ob, fps)
                        nc.sync.dma_start(
                            out=out_d[ts(et, 128), ts(lc, 512)], in_=ob
                        )

    nc.compile()
    return nc


def get_nc():
    global _nc_cache
    if _nc_cache is None:
        _nc_cache = build_nc()
    return _nc_cache


def make_core_inputs(x, Wq, Wk, Wv, Wo, bo):
    """Build the 8 per-core input maps from the full-size inputs."""
    x = np.asarray(x, F32)
    Wq = np.asarray(Wq, F32)
    Wk = np.asarray(Wk, F32)
    Wv = np.asarray(Wv, F32)
    Wo = np.asarray(Wo, F32)
    bo = np.asarray(bo, F32)

    xT_b = [np.ascontiguousarray(x[n].T).astype(BF16) for n in range(NB)]
    ones = np.ones((1, HPG * L), BF16)

    in_maps = []
    for c in range(NCORES):
        n, g = divmod(c, HPG)
        heads = [g * HPG + i for i in range(HPG)]

        wqk = np.empty((4, EMBED, 128), F32)
        for j in range(4):
            pair, qk = divmod(j, 2)
            hA = heads[2 * pair]
            hB = heads[2 * pair + 1]
            W = Wq if qk == 0 else Wk
            wqk[j, :, 0:HD] = W[hA * HD : (hA + 1) * HD, :].T
            wqk[j, :, HD:128] = W[hB * HD : (hB + 1) * HD, :].T

        wv = np.concatenate(
            [Wv[h * HD : (h + 1) * HD, :].T for h in heads], axis=1
        )  # [1024, 256]

        wo = np.zeros((HPG, ET, HD + 1, 128), F32)
        for lh, h in enumerate(heads):
            for et in range(ET):
                wo[lh, et, 0:HD, :] = Wo[
                    et * 128 : (et + 1) * 128, h * HD : (h + 1) * HD
                ].T
        for et in range(ET):
            wo[0, et, HD, :] = bo[et * 128 : (et + 1) * 128] / HPG

        in_maps.append(
            {
                "xT": xT_b[n],
                "wqk": wqk.astype(BF16),
                "wv": wv.astype(BF16),
                "wo": wo.astype(BF16),
                "ones": ones,
            }
        )
    return in_maps


def combine_outputs(results):
    """Sum the per-core fc_out partials and transpose back to [N, L, E]."""
    out = np.empty((NB, L, EMBED), F32)
    for n in range(NB):
        acc = results[n * HPG]["out"].astype(F32).copy()
        for g in range(1, HPG):
            acc += results[n * HPG + g]["out"]
        out[n] = acc.T
    return out


def kernel(x, Wq, Wk, Wv, Wo, bo):
    global LAST_EXEC_TIME_NS, LAST_RESULTS
    nc = get_nc()
    in_maps = make_core_inputs(x, Wq, Wk, Wv, Wo, bo)
    trace = bool(os.environ.get("KERNEL_TRACE"))
    kw = {}
    if trace:
        kw["trace"] = True
        kw["trace_cores"] = list(range(NCORES))
    res = run_bass_kernel_spmd(nc, in_maps, list(range(NCORES)), **kw)
    LAST_EXEC_TIME_NS = res.exec_time_ns
    LAST_RESULTS = res
    return combine_outputs(res.results)
